# revision 20
# baseline (speedup 1.0000x reference)
"""3-layer GAT (GATConv x3 + log_softmax) on 8 Trainium2 NeuronCores.

v3 strategy (dst-sharded, dma_gather edge phase, merged halves):
- Nodes permuted: core c owns new ids [c*6272, (c+1)*6272); 6250 real
  (degree-desc sorted) + 22 pad rows per core. Table rows are bf16 [128]
  payloads (premultiplied by the layer's W), AllGathered per layer into a
  Shared DRAM tensor that dma_gather reads directly.
- Edges dst-sharded; per 128-dst window, k-slot arrays split into lo/hi
  halves by src id (< / >= 25088) so gather indices fit int16. TWO
  dma_gathers per window write adjacent column ranges of ONE SBUF tile, so
  the whole DVE pipeline runs once per window on [128, K_lo+K_hi, 128].
- Padding slots point at a reserved pad row whose payload makes the
  attention score ~ -200 for every head, so exp() self-masks them.
- a_src is computed on device from the gathered payload; a_dst of own
  nodes comes out of the node transform. Layer 3 rows carry
  [h@W3 (9) | h@W3@as3 | h@W3@ad3], so its edge phase is cheap.
- k-axis reductions are in-place pairwise trees (contiguous adds); the
  strided transposed reduce is ~40x slower on DVE.
- Softmax without segment-max (scores are O(+-8)); denominators get +1e-6
  so pure-pad (zero-degree) nodes converge to the reference output.
"""
import numpy as np

import concourse.bass as bass
import concourse.mybir as mybir
import concourse.tile as tile
from concourse.masks import make_identity

# ---- problem constants (hardcoded per contest rules) ----
N = 50000
E = 800000
F_IN = 300
HEADS = 4
PH = 32
HID = 128
NCLS = 9
NEG = 0.2

NC_ = 8
RPER = 6250          # real nodes per core
NPER = 6272          # padded rows per core (49 * 128)
NPAD = NC_ * NPER    # 50176
HALF = NPAD // 2     # 25088 = 4 * NPER
P = 128
NWIN = NPER // P     # 49
PAD_LOC = 6271       # rebased pad row id in each half
B_PAD = 200.0        # pad-row score magnitude

f32 = mybir.dt.float32
bf16 = mybir.dt.bfloat16
i16 = mybir.dt.int16
AF = mybir.ActivationFunctionType
AX = mybir.AxisListType.X
MUL = mybir.AluOpType.mult
ADD = mybir.AluOpType.add
MAX = mybir.AluOpType.max


def _tree_reduce_k(nc, m, K):
    """In-place pairwise reduction over axis 1 of m [P, >=K, W]; result in
    m[:, 0, :]. Contiguous slice adds (fast) instead of a strided reduce."""
    k = K
    while k > 1:
        h = k // 2
        nc.vector.tensor_tensor(
            out=m[:, :h, :],
            in0=m[:, :h, :],
            in1=m[:, k - h : k, :],
            op=ADD,
        )
        k = k - h
    return m


# ----------------------------------------------------------------------------
# host-side graph prep
# ----------------------------------------------------------------------------
def prep_graph(edge_index):
    s = np.asarray(edge_index[0], dtype=np.int64)
    d = np.asarray(edge_index[1], dtype=np.int64)
    deg = np.bincount(d, minlength=N)

    old_of_new = np.full(NPAD, -1, dtype=np.int64)
    new_of_old = np.full(N, -1, dtype=np.int64)
    for c in range(NC_):
        nodes = np.arange(c * RPER, (c + 1) * RPER)
        order = nodes[np.argsort(-deg[nodes], kind="stable")]
        old_of_new[c * NPER : c * NPER + RPER] = order
        new_of_old[order] = c * NPER + np.arange(RPER)

    s_new = new_of_old[s]
    d_new = new_of_old[d]

    halves = []
    for X in range(2):
        msk = (s_new < HALF) if X == 0 else (s_new >= HALF)
        sX = s_new[msk] - X * HALF
        dX = d_new[msk]
        o = np.lexsort((sX, dX))
        sX, dX = sX[o], dX[o]
        starts = np.searchsorted(dX, np.arange(NPAD))
        ends = np.searchsorted(dX, np.arange(NPAD) + 1)
        halves.append((sX, starts, ends))

    # per (core, window, half): K
    Ks = np.zeros((2, NC_, NWIN), dtype=np.int64)
    for X in range(2):
        _, starts, ends = halves[X]
        cnt = ends - starts
        Ks[X] = cnt.reshape(NC_, NWIN, P).max(axis=2)
    Ku = Ks.max(axis=1)  # [2, NWIN] uniform across cores

    # per-call index stream: order [w0_lo, w0_hi, w1_lo, w1_hi, ...]
    coffs = []
    off = 0
    for w in range(NWIN):
        for X in range(2):
            coffs.append(off)
            off += 8 * int(Ku[X][w])
    totc = max(off, 16)

    idx_all = np.zeros((NC_, P, totc), dtype=np.int16)
    for c in range(NC_):
        ci = 0
        for w in range(NWIN):
            base = c * NPER + w * P
            for X in range(2):
                K = int(Ku[X][w])
                if K == 0:
                    ci += 1
                    continue
                sX, starts, ends = halves[X]
                a = np.full((P, K), PAD_LOC, dtype=np.int16)
                for p in range(P):
                    lo, hi = starts[base + p], ends[base + p]
                    a[p, : hi - lo] = sX[lo:hi]
                pos = a.T.ravel()  # position i = k*128 + p
                w16 = pos.reshape(-1, 16).T  # [16, cols]
                cols = w16.shape[1]
                idx_all[c, :, coffs[ci] : coffs[ci] + cols] = np.tile(w16, (8, 1))
                ci += 1

    return Ku, coffs, totc, idx_all, old_of_new, new_of_old


# ----------------------------------------------------------------------------
# post-build passes: library loads + ISA codegen + wait splitting
# ----------------------------------------------------------------------------
def insert_libs(nc):
    from concourse.library_config import all_libraries, standard
    import bass_rust

    m = {}
    for lib in all_libraries:
        for t in lib.instructions:
            m[t] = m.get(t, 0) | (1 << lib.index)
    bass_rust.insert_library_loads(nc, m, len(all_libraries), standard.index)


def finish_nc(nc, max_waits=1):
    import copy

    insert_libs(nc)
    mybir.codegen_inst_isa_subclasses(nc)

    n = 0
    for f in nc.m.functions:
        for blk in f.blocks:
            new_insts = []
            for ins in blk.instructions:
                is_isa = isinstance(ins, mybir.InstISA) or type(
                    ins
                ).__name__.startswith("InstISA")
                cap = 0 if is_isa else max_waits
                if ins.sync_info is not None and len(ins.sync_info.on_wait) > cap:
                    for w in list(ins.sync_info.on_wait):
                        noop = mybir.InstNoOp(
                            name=f"wsplit_{n}",
                            text_hint="wait_split",
                            bass_nofuse=True,
                        )
                        n += 1
                        noop.engine = ins.engine
                        si = copy.deepcopy(ins.sync_info)
                        si.on_update = type(si.on_update)()
                        si.on_wait = type(si.on_wait)([copy.deepcopy(w)])
                        noop.sync_info = si
                        new_insts.append(noop)
                    ins.sync_info.on_wait = type(ins.sync_info.on_wait)()
                new_insts.append(ins)
            if n:
                blk.instructions = new_insts
    return n


# ----------------------------------------------------------------------------
# device kernel builder
# ----------------------------------------------------------------------------
def build_nc(Ku, coffs, totc):
    KT = [int(Ku[0][w]) + int(Ku[1][w]) for w in range(NWIN)]
    KTMAX = max(KT)

    nc = bass.Bass()
    xT = nc.declare_dram_parameter("xT", [F_IN, NPER], f32, isOutput=False)
    w1aug = nc.declare_dram_parameter("w1aug", [F_IN, 132], f32, isOutput=False)
    w2aug = nc.declare_dram_parameter("w2aug", [P, 132], f32, isOutput=False)
    w3aug = nc.declare_dram_parameter("w3aug", [P, 11], f32, isOutput=False)
    asrep_p = nc.declare_dram_parameter("asrep", [P, 2 * P], bf16, isOutput=False)
    brep_p = nc.declare_dram_parameter("brep", [P, 2 * P + NCLS], f32, isOutput=False)
    pads_p = nc.declare_dram_parameter("pads", [3, HID], bf16, isOutput=False)
    idx_p = nc.declare_dram_parameter("idx", [P, totc], i16, isOutput=False)
    out_ext = nc.declare_dram_parameter("out", [NPER, NCLS], f32, isOutput=True)

    shard = [nc.dram_tensor(f"shard{l}", [NPER, HID], bf16) for l in range(3)]
    table = [
        nc.dram_tensor(f"table{l}", [NPAD, HID], bf16, addr_space="Shared")
        for l in range(3)
    ]
    rg = [list(range(NC_))]

    with tile.TileContext(nc) as tc:
        with (
            tc.tile_pool(name="resident", bufs=1) as rp,
            tc.tile_pool(name="sbuf", bufs=3) as pool,
            tc.tile_pool(name="gp", bufs=4) as gpool,
            tc.tile_pool(name="mp", bufs=2) as mpool,
            tc.tile_pool(name="psum", bufs=2, space="PSUM") as pp,
            tc.tile_pool(name="psum_t", bufs=2, space="PSUM") as ppt,
        ):
            # ---------- residents ----------
            idx_sb = rp.tile([P, totc], i16)
            nc.gpsimd.dma_start(out=idx_sb[:], in_=idx_p[:])
            ident = rp.tile([P, P], f32)
            make_identity(nc, ident[:])
            w1_sb = rp.tile([P, 3 * 132], f32)
            for kc in range(3):
                kd = min(P, F_IN - kc * P)
                nc.sync.dma_start(
                    out=w1_sb[:kd, kc * 132 : (kc + 1) * 132],
                    in_=w1aug[kc * P : kc * P + kd, :],
                )
            w2_sb = rp.tile([P, 132], f32)
            nc.sync.dma_start(out=w2_sb[:], in_=w2aug[:])
            w3_sb = rp.tile([P, 11], f32)
            nc.sync.dma_start(out=w3_sb[:], in_=w3aug[:])
            asrep_sb = rp.tile([P, 2 * P], bf16)
            nc.sync.dma_start(out=asrep_sb[:], in_=asrep_p[:])
            brep_sb = rp.tile([P, 2 * P + NCLS], f32)
            nc.sync.dma_start(out=brep_sb[:], in_=brep_p[:])
            adst = [rp.tile([P, NWIN * 4], f32, name=f"adst{l}") for l in range(3)]
            nreg = nc.gpsimd.alloc_register()

            # ---------- node transform: h1 = x @ W1 (+ a_dst1) ----------
            for t in range(NWIN):
                hp = pp.tile([P, 132], f32, tag="hp")
                for kc in range(3):
                    kd = min(P, F_IN - kc * P)
                    xt = pool.tile([P, P], f32, tag="xt")
                    nc.sync.dma_start(
                        out=xt[:kd, :],
                        in_=xT[kc * P : kc * P + kd, t * P : (t + 1) * P],
                    )
                    nc.tensor.matmul(
                        out=hp[:],
                        lhsT=xt[:kd, :],
                        rhs=w1_sb[:kd, kc * 132 : (kc + 1) * 132],
                        start=(kc == 0),
                        stop=(kc == 2),
                    )
                srow = pool.tile([P, HID], bf16, tag="srow")
                nc.scalar.activation(out=srow[:], in_=hp[:, 0:P], func=AF.Copy)
                nc.vector.tensor_copy(
                    out=adst[0][:, t * 4 : (t + 1) * 4], in_=hp[:, P : P + 4]
                )
                nc.sync.dma_start(out=shard[0][t * P : (t + 1) * P, :], in_=srow[:])
            nc.sync.dma_start(out=shard[0][NPER - 1 : NPER, :], in_=pads_p[0:1, :])

            # ---------- per-layer: allgather + edge phase ----------
            for l in range(3):
                nc.gpsimd.collective_compute(
                    "AllGather",
                    mybir.AluOpType.bypass,
                    ins=[shard[l][:]],
                    outs=[table[l][:]],
                    replica_groups=rg,
                )
                pay = HID if l < 2 else NCLS
                nh = HEADS if l < 2 else 1
                for w in range(NWIN):
                    Klo, Khi = int(Ku[0][w]), int(Ku[1][w])
                    K = Klo + Khi
                    gt = gpool.tile([P, KTMAX, HID], bf16, tag="g")
                    for X in range(2):
                        KX = Klo if X == 0 else Khi
                        if KX == 0:
                            continue
                        k0 = 0 if X == 0 else Klo
                        src = (
                            table[l][0:HALF, :]
                            if X == 0
                            else table[l][HALF:NPAD, :]
                        )
                        nidx = P * KX
                        ci = 2 * w + X
                        nc.gpsimd.reg_mov(nreg, nidx)
                        nc.gpsimd.dma_gather(
                            gt[:, k0 : k0 + KX, :],
                            src,
                            idx_sb[:, coffs[ci] : coffs[ci] + nidx // 16],
                            nidx,
                            nreg,
                            HID,
                            single_packet=False,
                        )
                    gv = gt[:, :K, :]
                    if l < 2:
                        # a_src = per-head dot(payload, as)
                        tmp = mpool.tile([P, KTMAX, HID], bf16, tag="m")
                        nc.vector.tensor_tensor(
                            out=tmp[:, :K, :],
                            in0=gv,
                            in1=asrep_sb[:, l * P : (l + 1) * P]
                            .unsqueeze(1)
                            .to_broadcast([P, K, HID]),
                            op=MUL,
                        )
                        asrc = pool.tile([P, KTMAX * 4], f32, tag="asrc")
                        nc.vector.reduce_sum(
                            out=asrc[:, : K * 4],
                            in_=tmp[:, :K, :].rearrange(
                                "p k (h c) -> p (k h) c", c=PH
                            ),
                            axis=AX,
                        )
                        e = pool.tile([P, KTMAX * 4], f32, tag="e")
                        nc.vector.tensor_tensor(
                            out=e[:, : K * 4].rearrange("p (k h) -> p k h", h=4),
                            in0=asrc[:, : K * 4].rearrange("p (k h) -> p k h", h=4),
                            in1=adst[l][:, w * 4 : w * 4 + 4]
                            .unsqueeze(1)
                            .to_broadcast([P, K, 4]),
                            op=ADD,
                        )
                        e2 = pool.tile([P, KTMAX * 4], f32, tag="e2")
                        nc.vector.tensor_scalar_mul(
                            out=e2[:, : K * 4], in0=e[:, : K * 4], scalar1=NEG
                        )
                        nc.vector.tensor_tensor(
                            out=e[:, : K * 4],
                            in0=e[:, : K * 4],
                            in1=e2[:, : K * 4],
                            op=MAX,
                        )
                        alpha = pool.tile([P, KTMAX * 4], bf16, tag="alpha")
                        nc.scalar.activation(
                            out=alpha[:, : K * 4], in_=e[:, : K * 4], func=AF.Exp
                        )
                        m = mpool.tile([P, KTMAX, HID], bf16, tag="m")
                        nc.vector.tensor_tensor(
                            out=m[:, :K, :].rearrange("p k (h c) -> p k h c", c=PH),
                            in0=gv.rearrange("p k (h c) -> p k h c", c=PH),
                            in1=alpha[:, : K * 4]
                            .rearrange("p (k h) -> p k h", h=4)
                            .unsqueeze(3)
                            .to_broadcast([P, K, 4, PH]),
                            op=MUL,
                        )
                        _tree_reduce_k(nc, m[:, :, :], K)
                        av = alpha[:, : K * 4].rearrange("p (k h) -> p k h", h=4)
                        _tree_reduce_k(nc, av, K)
                        num0 = m[:, 0, :]
                        den0 = alpha[:, 0:4]
                    else:
                        # layer 3: scores gathered directly (col 9)
                        e = pool.tile([P, KTMAX], f32, tag="e3")
                        nc.vector.tensor_tensor(
                            out=e[:, :K],
                            in0=gv[:, :, 9],
                            in1=adst[2][:, w * 4 : w * 4 + 1].to_broadcast([P, K]),
                            op=ADD,
                        )
                        e2 = pool.tile([P, KTMAX], f32, tag="e32")
                        nc.vector.tensor_scalar_mul(
                            out=e2[:, :K], in0=e[:, :K], scalar1=NEG
                        )
                        nc.vector.tensor_tensor(
                            out=e[:, :K], in0=e[:, :K], in1=e2[:, :K], op=MAX
                        )
                        alpha = pool.tile([P, KTMAX], bf16, tag="alpha3")
                        nc.scalar.activation(
                            out=alpha[:, :K], in_=e[:, :K], func=AF.Exp
                        )
                        m = mpool.tile([P, KTMAX, 16], bf16, tag="m3")
                        nc.vector.tensor_tensor(
                            out=m[:, :K, :],
                            in0=gv[:, :, 0:16],
                            in1=alpha[:, :K].unsqueeze(2).to_broadcast([P, K, 16]),
                            op=MUL,
                        )
                        _tree_reduce_k(nc, m[:, :, :], K)
                        av = alpha[:, :K].unsqueeze(2)
                        _tree_reduce_k(nc, av, K)
                        num0 = m[:, 0, :NCLS]
                        den0 = alpha[:, 0:1]
                    # normalize
                    dnS = pool.tile([P, HEADS], f32, tag="dnS")
                    nc.vector.tensor_scalar_add(
                        out=dnS[:, :nh], in0=den0, scalar1=1e-6
                    )
                    rdn = pool.tile([P, HEADS], f32, tag="rdn")
                    nc.vector.reciprocal(out=rdn[:, :nh], in_=dnS[:, :nh])
                    agg = pool.tile([P, pay], f32, tag="agg")
                    if l < 2:
                        nc.vector.tensor_tensor(
                            out=agg[:].rearrange("p (h c) -> p h c", h=4),
                            in0=num0.rearrange("p (h c) -> p h c", h=4),
                            in1=rdn[:, :4].unsqueeze(2).to_broadcast([P, 4, PH]),
                            op=MUL,
                        )
                    else:
                        nc.vector.tensor_tensor(
                            out=agg[:],
                            in0=num0,
                            in1=rdn[:, 0:1].to_broadcast([P, NCLS]),
                            op=MUL,
                        )
                    # bias + elu
                    boff = l * P
                    y = pool.tile([P, pay], f32, tag="y")
                    nc.vector.tensor_tensor(
                        out=y[:],
                        in0=agg[:],
                        in1=brep_sb[:, boff : boff + pay],
                        op=ADD,
                    )
                    neg = pool.tile([P, pay], f32, tag="neg")
                    nc.vector.tensor_scalar_min(out=neg[:], in0=y[:], scalar1=0.0)
                    en = pool.tile([P, pay], f32, tag="en")
                    nc.scalar.activation(out=en[:], in_=neg[:], func=AF.Exp)
                    pos = pool.tile([P, pay], f32, tag="pos")
                    nc.vector.tensor_scalar_max(out=pos[:], in0=y[:], scalar1=0.0)
                    elu = pool.tile([P, pay], f32, tag="elu")
                    nc.vector.tensor_add(out=elu[:], in0=pos[:], in1=en[:])
                    nc.vector.tensor_scalar_add(out=elu[:], in0=elu[:], scalar1=-1.0)
                    if l < 2:
                        # next-layer rows: elu @ Waug (premultiplied)
                        eluT_p = ppt.tile([P, P], f32, tag="tp")
                        nc.tensor.transpose(
                            out=eluT_p[:], in_=elu[:], identity=ident[:]
                        )
                        eluT = pool.tile([P, P], f32, tag="eluT")
                        nc.scalar.activation(
                            out=eluT[:], in_=eluT_p[:], func=AF.Copy
                        )
                        wa, wd = (w2_sb, 132) if l == 0 else (w3_sb, 11)
                        hp2 = pp.tile([P, 132], f32, tag="hp")
                        nc.tensor.matmul(
                            out=hp2[:, :wd],
                            lhsT=eluT[:],
                            rhs=wa[:, :wd],
                            start=True,
                            stop=True,
                        )
                        srow = pool.tile([P, HID], bf16, tag="srow")
                        if l == 0:
                            nc.scalar.activation(
                                out=srow[:], in_=hp2[:, 0:P], func=AF.Copy
                            )
                            nc.vector.tensor_copy(
                                out=adst[1][:, w * 4 : w * 4 + 4],
                                in_=hp2[:, P : P + 4],
                            )
                        else:
                            nc.vector.memset(srow[:], 0.0)
                            nc.scalar.activation(
                                out=srow[:, 0:11], in_=hp2[:, 0:11], func=AF.Copy
                            )
                            nc.vector.tensor_copy(
                                out=adst[2][:, w * 4 : w * 4 + 1],
                                in_=hp2[:, 10:11],
                            )
                        nc.sync.dma_start(
                            out=shard[l + 1][w * P : (w + 1) * P, :], in_=srow[:]
                        )
                        if w == NWIN - 1:
                            nc.sync.dma_start(
                                out=shard[l + 1][NPER - 1 : NPER, :],
                                in_=pads_p[l + 1 : l + 2, :],
                            )
                    else:
                        # elu then log_softmax over 9 classes
                        e9 = pool.tile([P, NCLS], f32, tag="e9")
                        nc.scalar.activation(out=e9[:], in_=elu[:], func=AF.Exp)
                        s9 = pool.tile([P, 1], f32, tag="s9")
                        nc.vector.reduce_sum(out=s9[:], in_=e9[:], axis=AX)
                        l9 = pool.tile([P, 1], f32, tag="l9")
                        nc.scalar.activation(out=l9[:], in_=s9[:], func=AF.Ln)
                        o9 = pool.tile([P, NCLS], f32, tag="o9")
                        nc.vector.tensor_tensor(
                            out=o9[:],
                            in0=elu[:],
                            in1=l9[:].to_broadcast([P, NCLS]),
                            op=mybir.AluOpType.subtract,
                        )
                        nc.sync.dma_start(
                            out=out_ext[w * P : (w + 1) * P, :], in_=o9[:]
                        )
    return nc


# ----------------------------------------------------------------------------
# host wrapper
# ----------------------------------------------------------------------------
def _np(x):
    return np.asarray(x)


def _bf16(a):
    import ml_dtypes

    return np.asarray(a, np.float32).astype(ml_dtypes.bfloat16)


def kernel(**inputs):
    from concourse.bass_utils import run_bass_kernel_spmd

    x = _np(inputs["x"]).astype(np.float32)
    edge_index = _np(inputs["edge_index"])
    W1 = _np(inputs["W1"]).astype(np.float32)
    as1 = _np(inputs["as1"]).astype(np.float32)
    ad1 = _np(inputs["ad1"]).astype(np.float32)
    b1 = _np(inputs["b1"]).astype(np.float32)
    W2 = _np(inputs["W2"]).astype(np.float32)
    as2 = _np(inputs["as2"]).astype(np.float32)
    ad2 = _np(inputs["ad2"]).astype(np.float32)
    b2 = _np(inputs["b2"]).astype(np.float32)
    W3 = _np(inputs["W3"]).astype(np.float32)
    as3 = _np(inputs["as3"]).astype(np.float32)
    ad3 = _np(inputs["ad3"]).astype(np.float32)
    b3 = _np(inputs["b3"]).astype(np.float32)

    Ku, coffs, totc, idx_all, old_of_new, new_of_old = prep_graph(edge_index)

    def blockdiag(a):  # [H, C] -> [H*C, H]
        H, C = a.shape
        out = np.zeros((H * C, H), np.float32)
        for h in range(H):
            out[h * C : (h + 1) * C, h] = a[h]
        return out

    w1aug = np.concatenate([W1, W1 @ blockdiag(ad1)], axis=1).astype(np.float32)
    w2aug = np.concatenate([W2, W2 @ blockdiag(ad2)], axis=1).astype(np.float32)
    w3aug = np.concatenate(
        [W3, (W3 @ as3[0])[:, None], (W3 @ ad3[0])[:, None]], axis=1
    ).astype(np.float32)

    asrep = np.concatenate(
        [np.tile(as1.ravel(), (P, 1)), np.tile(as2.ravel(), (P, 1))], axis=1
    )
    brep = np.concatenate(
        [np.tile(b1, (P, 1)), np.tile(b2, (P, 1)), np.tile(b3, (P, 1))], axis=1
    ).astype(np.float32)

    # pad rows: per-head payload that makes a_src == -B_PAD
    def pad_row(a):  # a [H, C]
        r = np.zeros(HID, np.float32)
        for h in range(a.shape[0]):
            blk = a[h]
            r[h * PH : (h + 1) * PH] = blk * (-B_PAD / (blk @ blk))
        return r

    pad3 = np.zeros(HID, np.float32)
    pad3[9] = -B_PAD
    pads = np.stack([pad_row(as1), pad_row(as2), pad3]).astype(np.float32)

    xT = np.zeros((F_IN, NPAD), np.float32)
    real = old_of_new >= 0
    xT[:, real] = x[old_of_new[real]].T

    nc = build_nc(Ku, coffs, totc)
    in_maps = []
    for c in range(NC_):
        in_maps.append(
            {
                "xT": np.ascontiguousarray(xT[:, c * NPER : (c + 1) * NPER]),
                "w1aug": w1aug,
                "w2aug": w2aug,
                "w3aug": w3aug,
                "asrep": _bf16(asrep),
                "brep": brep,
                "pads": _bf16(pads),
                "idx": idx_all[c],
            }
        )
    finish_nc(nc)
    res = run_bass_kernel_spmd(nc, in_maps, list(range(NC_)))
    out = np.zeros((N, NCLS), np.float32)
    for c in range(NC_):
        rows = old_of_new[c * NPER : (c + 1) * NPER]
        m = rows >= 0
        out[rows[m]] = res.results[c]["out"][m]
    return out
